# revision 27
# baseline (speedup 1.0000x reference)
"""Trainium2 8-core Bass kernel for nn_AttentionFlow (GNN message passing).

Strategy (per core c of 8):
  - Phase A (device): hc = tanh(hidden_con @ Wc + bc) and
    hu = tanh(hidden_uncon @ Wu + bu), row-sharded across the 8 cores.
    Host pre-transposes the inputs, so the device computes hc^T / hu^T
    directly: out[d', n] = sum_d W[d, d'] X^T[d, n] with the bias folded
    into the tanh activation (per-partition bias = d').  Two 512-column
    chunks share one [128, 512] PSUM tile (partition halves).
  - Host: gathers per-edge features from the phase-A tables
    (hc[e2vi], hc[e2vj], hu[vj], hu[vi_seg], ABCD[rel]) and packs them
    field-contiguously into a [10, 128, 2*FW] bf16 tensor per core (pure
    data movement / index math, no per-edge float compute).
  - Phase B (device): 8 streaming DMAs per 256-segment supertile (no
    indirect DMAs), one full SBUF tile per field so every DVE op runs on
    fully-packed [128, 40, 64] bf16 operands (fast path; sliced views of
    a larger tile run ~2x slower).  The F layer on DVE:
      x = f0*(f3*A + f4*B) + f1*(f3*C + f4*D)
    with ABCD[r] = [ws0+ws1*rel | ws2+ws3*rel | ws4+ws5*rel | ws6+ws7*rel]
    * |out_w| built host-side from the (tiny) parameter tables.
    logits = sum_d sign(w_d) relu(x_d): host permutes the d axis so
    positive-sign dims are contiguous -> two free-dim reduces.  Segment
    softmax (one segment of 20 edges per partition-row slot) is batched
    over all tiles at the end.  GpSimd is kept idle on purpose: its
    elementwise ops steal the shared DVE SBUF port and stall DVE.
  - Edges are sharded 50000/core, aligned to the 20-edge vi-segment
    structure, so the softmax is fully core-local.
  - Host: final (eg, vj) scatter-add of the per-edge trans_att partials.
"""

import sys

sys.path.insert(0, "/opt/trn_rl_repo")

import numpy as np
import ml_dtypes

from concourse import bass, bacc, mybir
import concourse.tile as tile
from concourse.bass_utils import run_bass_kernel_spmd

BF = ml_dtypes.bfloat16

NCORES = 8
B = 4
E = 400_000
EPC = E // NCORES            # 50000 edges per core
KK = 20                      # edges per vi segment
SEGS = EPC // KK             # 2500 segments per core
P = 128
NT = (SEGS + P - 1) // P     # 20 tiles of 128 segments
NST = NT // 2                # 10 supertiles of 256 segments
SEG_PAD = NT * P             # 2560
NN = 50_000
NREL = 500
D = 64
DLG = 256
NMEM = 131_072
HC_SH = NMEM // NCORES       # 16384 hidden_con rows per core
HU_SH = 7_168                # hidden_uncon rows per core (8*7168=57344)
HU_PAD = HU_SH * NCORES
FW = 9_024                   # feat cols: f0|f3|f4|A|B|C|D (7*1280) + f1 (64)

f32 = mybir.dt.float32
bf16 = mybir.dt.bfloat16


def _unblock_pairsT(buf, rows):
    """Device stores [128, 1024] bf16 tiles holding two 1024-row blocks
    (4 chunks of 512 cols of X^T; chunk = 4*B2 + 2*b + h, d on partition
    h*64+d).  Rebuild X^T [D, rows]."""
    n2 = rows // 2048
    b6 = buf.reshape(n2, 2, D, 2, 512)            # [B2, h, d, b, q]
    xT = b6.transpose(0, 3, 1, 2, 4).reshape(n2 * 4, D, 512)
    return xT.transpose(1, 0, 2).reshape(D, rows)


def _unblock_hc(buf):
    return _unblock_pairsT(buf, HC_SH).T


def _unblock_hu(buf):
    main = _unblock_pairsT(buf[:6144 * D], 6144)  # [64, 6144]
    tail = buf[6144 * D:].reshape(2, D, 512)      # [h, d, q]
    xT = np.concatenate([main, tail[0], tail[1]], axis=1)
    return xT.T                                    # [7168, 64]


def _build_proj():
    """Phase A: sharded hc/hu projections, transposed-output form."""
    nc = bacc.Bacc("TRN2", target_bir_lowering=False, debug=False,
                   num_devices=NCORES)
    hconT = nc.declare_dram_parameter("hconT", [D, HC_SH], bf16,
                                      isOutput=False)
    huT = nc.declare_dram_parameter("huT", [DLG, HU_SH], bf16, isOutput=False)
    wc_p = nc.declare_dram_parameter("wc_p", [D, D], bf16, isOutput=False)
    wu_p = nc.declare_dram_parameter("wu_p", [DLG, D], bf16, isOutput=False)
    b_p = nc.declare_dram_parameter("b_p", [P, 2], f32, isOutput=False)
    hc_out = nc.declare_dram_parameter("hc_sh", [HC_SH * D], bf16,
                                       isOutput=True)
    hu_out = nc.declare_dram_parameter("hu_sh", [HU_SH * D], bf16,
                                       isOutput=True)

    NCH = 4                       # input-load chunks for overlap
    CHC = HC_SH // NCH            # 4096
    CHU = HU_SH // NCH            # 1792

    with tile.TileContext(nc) as tc:
        with (
            tc.tile_pool(name="const", bufs=1) as cpool,
            tc.tile_pool(name="proj", bufs=3) as ppool,
            tc.tile_pool(name="psum", bufs=3, space="PSUM") as pspool,
        ):
            # weights/bias pre-cast to bf16 on the host: all loads go over
            # the two HWDGE rings (sync + scalar), no SWDGE involved
            wc_sb = cpool.tile([D, D], bf16)
            nc.scalar.dma_start(out=wc_sb[:], in_=wc_p[:])
            bb = cpool.tile([P, 2], f32)
            nc.scalar.dma_start(out=bb[:], in_=b_p[:])
            wu_sb = cpool.tile([DLG // 2, 2, D], bf16)
            nc.scalar.dma_start(out=wu_sb[:, 0, :], in_=wu_p[0:128, :])
            nc.scalar.dma_start(out=wu_sb[:, 1, :], in_=wu_p[128:256, :])

            # xt streams on the sync ring while xu streams on the scalar
            # ring; measured faster than sequencing both on one ring
            xt = cpool.tile([D, HC_SH], bf16)
            xu = cpool.tile([DLG // 2, 2, HU_SH], bf16)
            for ch in range(NCH):
                nc.sync.dma_start(out=xt[:, ch * CHC:(ch + 1) * CHC],
                                  in_=hconT[:, ch * CHC:(ch + 1) * CHC])
                nc.scalar.dma_start(out=xu[:, 0, ch * CHU:(ch + 1) * CHU],
                                    in_=huT[0:128, ch * CHU:(ch + 1) * CHU])
                nc.scalar.dma_start(out=xu[:, 1, ch * CHU:(ch + 1) * CHU],
                                    in_=huT[128:256, ch * CHU:(ch + 1) * CHU])

            def proj_block(blk, mms, bias, outbuf, ot2, half):
                ps = pspool.tile([P, 512], f32, space="PSUM", tag="ps")
                for h in range(2):
                    o = blk * 1024 + h * 512
                    for i, (lhs, rhs) in enumerate(mms):
                        nc.tensor.matmul(out=ps[h * D:(h + 1) * D, :],
                                         lhsT=lhs, rhs=rhs[:, o:o + 512],
                                         start=(i == 0),
                                         stop=(i == len(mms) - 1))
                nc.scalar.activation(out=ot2[:, half * 512:(half + 1) * 512],
                                     in_=ps[:],
                                     func=mybir.ActivationFunctionType.Tanh,
                                     bias=bias, scale=1.0)

            for blk2 in range(HC_SH // 2048):
                ot2 = ppool.tile([P, 1024], bf16, tag="ot2")
                for half in range(2):
                    proj_block(2 * blk2 + half, [(wc_sb[:], xt)],
                               bb[:, 0:1], hc_out, ot2, half)
                nc.sync.dma_start(
                    out=hc_out[blk2 * 2048 * D:(blk2 + 1) * 2048 * D],
                    in_=ot2[:])

            hu_mms = [(wu_sb[:, 0, :], xu[:, 0, :]), (wu_sb[:, 1, :], xu[:, 1, :])]
            for blk2 in range(HU_SH // 2048):
                ot2 = ppool.tile([P, 1024], bf16, tag="ot2")
                for half in range(2):
                    proj_block(2 * blk2 + half, hu_mms,
                               bb[:, 1:2], hu_out, ot2, half)
                nc.sync.dma_start(
                    out=hu_out[blk2 * 2048 * D:(blk2 + 1) * 2048 * D],
                    in_=ot2[:])
            # odd tail block of hu (7168 = 3*2048 + 1024)
            ot2 = ppool.tile([P, 1024], bf16, tag="ot2")
            proj_block(HU_SH // 1024 - 1, hu_mms, bb[:, 1:2], hu_out, ot2, 0)
            nc.sync.dma_start(
                out=hu_out[(HU_SH - 1024) * D:HU_SH * D],
                in_=ot2[:, 0:512])
    nc.finalize()
    return nc


def _build_main(dp):
    """Phase B: streaming F-layer + batched segment softmax.
    feat is field-contiguous per supertile: every elementwise op runs on a
    flat [P, 2560] bf16 slice (DVE 2x mode); one product per supertile is
    placed on the otherwise-idle GpSimd engine."""
    nc = bacc.Bacc("TRN2", target_bir_lowering=False, debug=False,
                   num_devices=NCORES)
    FW2 = 2 * FW
    W2 = 2 * KK * D              # 2560: one field's width per supertile
    feat = nc.declare_dram_parameter("feat", [NST, P, FW2], bf16,
                                     isOutput=False)
    meta = nc.declare_dram_parameter("meta", [P, NT, 21], f32,
                                     isOutput=False)
    ta_ext = nc.declare_dram_parameter("ta", [P, NT * KK], f32, isOutput=True)

    S2 = 2

    with tile.TileContext(nc) as tc:
        with (
            tc.tile_pool(name="const", bufs=1) as cpool,
            tc.tile_pool(name="ld", bufs=4) as gpool,
            tc.tile_pool(name="mid", bufs=2) as mpool,
            tc.tile_pool(name="sm", bufs=2) as spool,
        ):
            logit_all = cpool.tile([P, NT, KK], f32)
            meta_sb = cpool.tile([P, NT, 21], f32)

            TT = nc.vector.tensor_tensor
            MU = mybir.AluOpType.mult
            AD = mybir.AluOpType.add

            K2 = S2 * KK             # 40 segment-slots per supertile

            for st in range(NST):
                # one full SBUF tile per field: fully-packed [P, 40, 64]
                # operands hit the fast DVE path (sliced views do not).
                # Loads ordered by first use in the op chain below.
                fts = {}
                for i in (1, 3, 2, 4, 5, 6, 0):
                    t = gpool.tile([P, W2], bf16, tag=f"fld{i}")
                    nc.sync.dma_start(out=t[:],
                                      in_=feat[st][:, i * W2:(i + 1) * W2])
                    fts[i] = t
                f1t = gpool.tile([P, S2 * D], bf16, tag="f1")
                nc.sync.dma_start(
                    out=f1t[:], in_=feat[st][:, 7 * W2:7 * W2 + S2 * D])
                if st == 0:
                    # meta is consumed only by the final softmax; keep it
                    # off the first supertile's critical load path
                    nc.sync.dma_start(out=meta_sb[:], in_=meta[:])

                def v3(t):
                    return t[:].rearrange("p (k d) -> p k d", d=D)

                f0, f3, f4, Av, Bv, Cv, Dv = (v3(fts[i]) for i in range(7))
                f1b = f1t[:].rearrange("p (s o d) -> p s o d", s=S2,
                                       d=D).to_broadcast([P, S2, KK, D])

                u1 = mpool.tile([P, W2], bf16, tag="u1")
                TT(out=v3(u1), in0=f3, in1=Av, op=MU)
                u2 = mpool.tile([P, W2], bf16, tag="u2")
                TT(out=v3(u2), in0=f4, in1=Bv, op=MU)
                u3 = mpool.tile([P, W2], bf16, tag="u3")
                TT(out=v3(u3), in0=f3, in1=Cv, op=MU)
                u4 = mpool.tile([P, W2], bf16, tag="u4")
                TT(out=v3(u4), in0=f4, in1=Dv, op=MU)
                TT(out=v3(u1), in0=v3(u1), in1=v3(u2), op=AD)
                TT(out=v3(u3), in0=v3(u3), in1=v3(u4), op=AD)
                TT(out=v3(u2), in0=v3(u1), in1=f0, op=MU)
                TT(out=u4[:].rearrange("p (s k d) -> p s k d", s=S2, d=D),
                   in0=u3[:].rearrange("p (s k d) -> p s k d", s=S2, d=D),
                   in1=f1b, op=MU)
                # x reuses u1's buffer, xr reuses u3's (both dead here)
                x = mpool.tile([P, W2], bf16, tag="u1")
                TT(out=v3(x), in0=v3(u2), in1=v3(u4), op=AD)
                xr = mpool.tile([P, W2], bf16, tag="u3")
                nc.scalar.activation(out=xr[:], in_=x[:],
                                     func=mybir.ActivationFunctionType.Relu)

                xr4 = xr[:].rearrange("p (s k d) -> p s k d", s=S2, d=D)
                lsl = logit_all[:, 2 * st:2 * st + 2, :]
                if dp == D:
                    nc.vector.tensor_reduce(out=lsl, in_=xr4,
                                            axis=mybir.AxisListType.X, op=AD)
                elif dp == 0:
                    neg = spool.tile([P, S2, KK], f32, tag="neg")
                    nc.vector.tensor_reduce(out=neg[:], in_=xr4,
                                            axis=mybir.AxisListType.X, op=AD)
                    nc.vector.tensor_scalar_mul(lsl, neg[:], -1.0)
                else:
                    pos = spool.tile([P, S2, KK], f32, tag="pos")
                    nc.vector.tensor_reduce(out=pos[:], in_=xr4[:, :, :, 0:dp],
                                            axis=mybir.AxisListType.X, op=AD)
                    neg = spool.tile([P, S2, KK], f32, tag="neg")
                    nc.vector.tensor_reduce(out=neg[:], in_=xr4[:, :, :, dp:D],
                                            axis=mybir.AxisListType.X, op=AD)
                    nc.vector.scalar_tensor_tensor(
                        out=lsl, in0=pos[:], scalar=1.0, in1=neg[:],
                        op0=MU, op1=mybir.AluOpType.subtract)

            # batched segment softmax + attention weighting over all tiles
            m = spool.tile([P, NT], f32, tag="m")
            nc.vector.tensor_reduce(out=m[:], in_=logit_all[:],
                                    axis=mybir.AxisListType.X,
                                    op=mybir.AluOpType.max)
            mb = m[:].rearrange("p (t o) -> p t o", o=1)
            TT(out=logit_all[:], in0=logit_all[:],
               in1=mb.to_broadcast([P, NT, KK]),
               op=mybir.AluOpType.subtract)
            ex = cpool.tile([P, NT, KK], f32)
            nc.scalar.activation(out=ex[:], in_=logit_all[:],
                                 func=mybir.ActivationFunctionType.Exp)
            den = spool.tile([P, NT], f32, tag="den")
            nc.vector.tensor_reduce(out=den[:], in_=ex[:],
                                    axis=mybir.AxisListType.X, op=AD)
            rec = spool.tile([P, NT], f32, tag="rec")
            nc.vector.reciprocal(rec[:], den[:])
            sc = spool.tile([P, NT], f32, tag="sc")
            TT(out=sc[:], in0=rec[:],
               in1=meta_sb[:, :, 20:21].rearrange("p t o -> p (t o)"), op=MU)
            TT(out=ex[:], in0=ex[:], in1=meta_sb[:, :, 0:20], op=MU)
            scb = sc[:].rearrange("p (t o) -> p t o", o=1)
            TT(out=ex[:], in0=ex[:], in1=scb.to_broadcast([P, NT, KK]), op=MU)

            nc.sync.dma_start(
                out=ta_ext[:],
                in_=ex[:].rearrange("p a b -> p (a b)"))
    nc.finalize()
    return nc


_CACHE = {}


def _prep(inputs):
    """Host-side: permute the d axis by out_w sign, transpose/shard the
    projection inputs (pure data movement + integer index math)."""
    na = np.asarray(inputs["node_attention"], np.float32)
    se = np.asarray(inputs["scanned_edges"])
    ey = np.asarray(inputs["edges_y"], np.float32)
    huncon = np.asarray(inputs["hidden_uncon"], np.float32)[0]
    hcon = np.asarray(inputs["hidden_con"], np.float32)
    Wc = np.asarray(inputs["Wc"], np.float32)
    bc = np.asarray(inputs["bc"], np.float32)
    Wu = np.asarray(inputs["Wu"], np.float32)
    bu = np.asarray(inputs["bu"], np.float32)
    relt = np.asarray(inputs["rel_table"], np.float32)
    ws = np.asarray(inputs["ws"], np.float32)
    fb = np.asarray(inputs["fb"], np.float32)
    out_w = np.asarray(inputs["out_w"], np.float32)

    # d-permutation: positive out_w dims first
    perm = np.argsort(out_w <= 0, kind="stable")
    dp = int((out_w > 0).sum())
    Wcp = np.ascontiguousarray(Wc[:, perm]).astype(BF)
    Wup = np.ascontiguousarray(Wu[:, perm]).astype(BF)
    bp = np.empty((P, 2), np.float32)
    bp[0:D, 0] = bp[D:P, 0] = bc[perm]
    bp[0:D, 1] = bp[D:P, 1] = bu[perm]
    assert not np.any(fb != 0), "fb != 0 unsupported by this build"

    # fused per-rel tables ABCD[r] = [ws0+ws1*rel | ws2+ws3*rel |
    # ws4+ws5*rel | ws6+ws7*rel] * |out_w|  (parameter-table prep)
    wsp = ws[:, perm]
    absw = np.abs(out_w[perm])[None]
    rp = relt[:, perm]
    gtab = np.concatenate(
        [(wsp[2 * t] + wsp[2 * t + 1] * rp) * absw for t in range(4)],
        axis=1).astype(BF)                                       # [500, 256]

    eg, vi, vj, rel = (se[:, i].astype(np.int64) for i in range(4))
    e2vi, e2vj = se[:, 6].astype(np.int64), se[:, 7].astype(np.int64)

    hu_pad = np.zeros((HU_PAD, DLG), np.float32)
    hu_pad[:NN] = huncon
    in_maps_a = []
    for c in range(NCORES):
        hcT = np.ascontiguousarray(
            hcon[c * HC_SH:(c + 1) * HC_SH].T).astype(BF)
        huT = np.ascontiguousarray(
            hu_pad[c * HU_SH:(c + 1) * HU_SH].T).astype(BF)
        in_maps_a.append({"hconT": hcT, "huT": huT,
                          "wc_p": Wcp, "wu_p": Wup, "b_p": bp})
    return in_maps_a, dp, gtab, (na, eg, vi, vj, rel, e2vi, e2vj, ey)


def _pack_feats(hc_full, hu_full, gtab, host):
    """Host-side per-edge gather + packing into per-core feat/meta."""
    na, eg, vi, vj, rel, e2vi, e2vj, ey = host
    in_maps_b = []
    for c in range(NCORES):
        s = c * EPC
        fv = np.zeros((NST, P, 2 * FW), BF)

        def setf(off2, arr, w=KK * D):
            # sub-block s2 of field at off2 holds segment (2*st+s2)*128+p
            padded = np.zeros((SEG_PAD, w), BF)
            padded[:arr.shape[0]] = arr
            fv[:, :, off2:off2 + 2 * w] = padded.reshape(
                NST, 2, P, w).transpose(0, 2, 1, 3).reshape(NST, P, 2 * w)

        W1 = KK * D
        setf(0 * 2 * W1, hc_full[e2vi[s:s + EPC]].reshape(SEGS, W1))
        setf(1 * 2 * W1, hc_full[e2vj[s:s + EPC]].reshape(SEGS, W1))
        setf(2 * 2 * W1, hu_full[vj[s:s + EPC]].reshape(SEGS, W1))
        g_all = gtab[rel[s:s + EPC]]                 # [EPC, 256]
        for i in range(4):
            setf((3 + i) * 2 * W1,
                 np.ascontiguousarray(
                     g_all[:, i * D:(i + 1) * D]).reshape(SEGS, W1))
        setf(7 * 2 * W1, hu_full[vi[s:s + EPC][::KK]], w=D)

        mt = np.zeros((P, NT, 21), np.float32)
        eyp = np.zeros((SEG_PAD, KK), np.float32)
        eyp[:SEGS] = ey[s:s + EPC].reshape(SEGS, KK)
        mt[:, :, 0:20] = eyp.reshape(NT, P, KK).transpose(1, 0, 2)
        nav = np.zeros(SEG_PAD, np.float32)
        nav[:SEGS] = na[c // 2, vi[s:s + EPC][::KK]]
        mt[:, :, 20] = nav.reshape(NT, P).T
        in_maps_b.append({"feat": fv, "meta": mt})
    return in_maps_b


def kernel(**inputs):
    in_maps_a, dp, gtab, host = _prep(inputs)
    if "proj" not in _CACHE:
        _CACHE["proj"] = _build_proj()
    key = ("main", dp)
    if key not in _CACHE:
        _CACHE[key] = _build_main(dp)

    resA = run_bass_kernel_spmd(_CACHE["proj"], in_maps_a,
                                core_ids=list(range(NCORES)))
    hc_full = np.concatenate(
        [_unblock_hc(np.asarray(r["hc_sh"])) for r in resA.results], 0)
    hu_full = np.concatenate(
        [_unblock_hu(np.asarray(r["hu_sh"])) for r in resA.results], 0)

    in_maps_b = _pack_feats(hc_full, hu_full, gtab, host)
    resB = run_bass_kernel_spmd(_CACHE[key], in_maps_b,
                                core_ids=list(range(NCORES)))
    na, eg, vi, vj, rel, e2vi, e2vj, ey = host
    out = np.zeros((B, NN), np.float32)
    for c in range(NCORES):
        ta = np.asarray(resB.results[c]["ta"]).reshape(P, NT, KK)
        ta_edges = ta.transpose(1, 0, 2).reshape(-1)[:EPC]
        s = c * EPC
        np.add.at(out, (eg[s:s + EPC], vj[s:s + EPC]), ta_edges)
    return out


# revision 28
# speedup vs baseline: 1.1553x; 1.1553x over previous
"""Trainium2 8-core Bass kernel for nn_AttentionFlow (GNN message passing).

Strategy (per core c of 8):
  - Phase A (device): hc = tanh(hidden_con @ Wc + bc) and
    hu = tanh(hidden_uncon @ Wu + bu), row-sharded across the 8 cores.
    Host pre-transposes the inputs, so the device computes hc^T / hu^T
    directly: out[d', n] = sum_d W[d, d'] X^T[d, n] with the bias folded
    into the tanh activation (per-partition bias = d').  Two 512-column
    chunks share one [128, 512] PSUM tile (partition halves).
  - Host: gathers per-edge features from the phase-A tables
    (hc[e2vi], hc[e2vj], hu[vj], hu[vi_seg], ABCD[rel]) and packs them
    field-contiguously into a [10, 128, 2*FW] bf16 tensor per core (pure
    data movement / index math, no per-edge float compute).
  - Phase B (device): 8 streaming DMAs per 256-segment supertile (no
    indirect DMAs), one full SBUF tile per field so every DVE op runs on
    fully-packed [128, 40, 64] bf16 operands (fast path; sliced views of
    a larger tile run ~2x slower).  The F layer on DVE:
      x = f0*(f3*A + f4*B) + f1*(f3*C + f4*D)
    with ABCD[r] = [ws0+ws1*rel | ws2+ws3*rel | ws4+ws5*rel | ws6+ws7*rel]
    * |out_w| built host-side from the (tiny) parameter tables.
    logits = sum_d sign(w_d) relu(x_d): host permutes the d axis so
    positive-sign dims are contiguous -> two free-dim reduces.  Segment
    softmax (one segment of 20 edges per partition-row slot) is batched
    over all tiles at the end.  GpSimd is kept idle on purpose: its
    elementwise ops steal the shared DVE SBUF port and stall DVE.
  - Edges are sharded 50000/core, aligned to the 20-edge vi-segment
    structure, so the softmax is fully core-local.
  - Host: final (eg, vj) scatter-add of the per-edge trans_att partials.
"""

import sys

sys.path.insert(0, "/opt/trn_rl_repo")

import numpy as np
import ml_dtypes

from concourse import bass, bacc, mybir
import concourse.tile as tile
from concourse.bass_utils import run_bass_kernel_spmd

BF = ml_dtypes.bfloat16

NCORES = 8
B = 4
E = 400_000
EPC = E // NCORES            # 50000 edges per core
KK = 20                      # edges per vi segment
SEGS = EPC // KK             # 2500 segments per core
P = 128
NT = (SEGS + P - 1) // P     # 20 tiles of 128 segments
NST = NT // 2                # 10 supertiles of 256 segments
SEG_PAD = NT * P             # 2560
NN = 50_000
NREL = 500
D = 64
DLG = 256
NMEM = 131_072
HC_SH = NMEM // NCORES       # 16384 hidden_con rows per core
HU_SH = 7_168                # hidden_uncon rows per core (8*7168=57344)
HU_PAD = HU_SH * NCORES
FW = 9_024                   # feat cols: f0|f3|f4|A|B|C|D (7*1280) + f1 (64)

f32 = mybir.dt.float32
bf16 = mybir.dt.bfloat16


def _unblock_pairsT(buf, rows):
    """Device stores [128, 1024] bf16 tiles holding two 1024-row blocks
    (4 chunks of 512 cols of X^T; chunk = 4*B2 + 2*b + h, d on partition
    h*64+d).  Rebuild X^T [D, rows]."""
    n2 = rows // 2048
    b6 = buf.reshape(n2, 2, D, 2, 512)            # [B2, h, d, b, q]
    xT = b6.transpose(0, 3, 1, 2, 4).reshape(n2 * 4, D, 512)
    return xT.transpose(1, 0, 2).reshape(D, rows)


def _unblock_hc(buf):
    return _unblock_pairsT(buf, HC_SH).T


def _unblock_hu(buf):
    main = _unblock_pairsT(buf[:6144 * D], 6144)  # [64, 6144]
    tail = buf[6144 * D:].reshape(2, D, 512)      # [h, d, q]
    xT = np.concatenate([main, tail[0], tail[1]], axis=1)
    return xT.T                                    # [7168, 64]


def _build_proj():
    """Phase A: sharded hc/hu projections, transposed-output form."""
    nc = bacc.Bacc("TRN2", target_bir_lowering=False, debug=False,
                   num_devices=NCORES)
    hconT = nc.declare_dram_parameter("hconT", [D, HC_SH], bf16,
                                      isOutput=False)
    huT = nc.declare_dram_parameter("huT", [DLG, HU_SH], bf16, isOutput=False)
    wc_p = nc.declare_dram_parameter("wc_p", [D, D], bf16, isOutput=False)
    wu_p = nc.declare_dram_parameter("wu_p", [DLG, D], bf16, isOutput=False)
    b_p = nc.declare_dram_parameter("b_p", [P, 2], f32, isOutput=False)
    hc_out = nc.declare_dram_parameter("hc_sh", [HC_SH * D], bf16,
                                       isOutput=True)
    hu_out = nc.declare_dram_parameter("hu_sh", [HU_SH * D], bf16,
                                       isOutput=True)

    NCH = 4                       # input-load chunks for overlap
    CHC = HC_SH // NCH            # 4096
    CHU = HU_SH // NCH            # 1792

    with tile.TileContext(nc) as tc:
        with (
            tc.tile_pool(name="const", bufs=1) as cpool,
            tc.tile_pool(name="proj", bufs=3) as ppool,
            tc.tile_pool(name="psum", bufs=3, space="PSUM") as pspool,
        ):
            # weights/bias pre-cast to bf16 on the host: all loads go over
            # the two HWDGE rings (sync + scalar), no SWDGE involved
            wc_sb = cpool.tile([D, D], bf16)
            nc.scalar.dma_start(out=wc_sb[:], in_=wc_p[:])
            bb = cpool.tile([P, 2], f32)
            nc.scalar.dma_start(out=bb[:], in_=b_p[:])
            wu_sb = cpool.tile([DLG // 2, 2, D], bf16)
            nc.scalar.dma_start(out=wu_sb[:, 0, :], in_=wu_p[0:128, :])
            nc.scalar.dma_start(out=wu_sb[:, 1, :], in_=wu_p[128:256, :])

            # xt streams on the sync ring while xu streams on the scalar
            # ring; measured faster than sequencing both on one ring
            xt = cpool.tile([D, HC_SH], bf16)
            xu = cpool.tile([DLG // 2, 2, HU_SH], bf16)
            for ch in range(NCH):
                nc.sync.dma_start(out=xt[:, ch * CHC:(ch + 1) * CHC],
                                  in_=hconT[:, ch * CHC:(ch + 1) * CHC])
                nc.scalar.dma_start(out=xu[:, 0, ch * CHU:(ch + 1) * CHU],
                                    in_=huT[0:128, ch * CHU:(ch + 1) * CHU])
                nc.scalar.dma_start(out=xu[:, 1, ch * CHU:(ch + 1) * CHU],
                                    in_=huT[128:256, ch * CHU:(ch + 1) * CHU])

            def proj_block(blk, mms, bias, outbuf, ot2, half):
                ps = pspool.tile([P, 512], f32, space="PSUM", tag="ps")
                for h in range(2):
                    o = blk * 1024 + h * 512
                    for i, (lhs, rhs) in enumerate(mms):
                        nc.tensor.matmul(out=ps[h * D:(h + 1) * D, :],
                                         lhsT=lhs, rhs=rhs[:, o:o + 512],
                                         start=(i == 0),
                                         stop=(i == len(mms) - 1))
                nc.scalar.activation(out=ot2[:, half * 512:(half + 1) * 512],
                                     in_=ps[:],
                                     func=mybir.ActivationFunctionType.Tanh,
                                     bias=bias, scale=1.0)

            for blk2 in range(HC_SH // 2048):
                ot2 = ppool.tile([P, 1024], bf16, tag="ot2")
                for half in range(2):
                    proj_block(2 * blk2 + half, [(wc_sb[:], xt)],
                               bb[:, 0:1], hc_out, ot2, half)
                nc.sync.dma_start(
                    out=hc_out[blk2 * 2048 * D:(blk2 + 1) * 2048 * D],
                    in_=ot2[:])

            hu_mms = [(wu_sb[:, 0, :], xu[:, 0, :]), (wu_sb[:, 1, :], xu[:, 1, :])]
            for blk2 in range(HU_SH // 2048):
                ot2 = ppool.tile([P, 1024], bf16, tag="ot2")
                for half in range(2):
                    proj_block(2 * blk2 + half, hu_mms,
                               bb[:, 1:2], hu_out, ot2, half)
                nc.sync.dma_start(
                    out=hu_out[blk2 * 2048 * D:(blk2 + 1) * 2048 * D],
                    in_=ot2[:])
            # odd tail block of hu (7168 = 3*2048 + 1024)
            ot2 = ppool.tile([P, 1024], bf16, tag="ot2")
            proj_block(HU_SH // 1024 - 1, hu_mms, bb[:, 1:2], hu_out, ot2, 0)
            nc.sync.dma_start(
                out=hu_out[(HU_SH - 1024) * D:HU_SH * D],
                in_=ot2[:, 0:512])
    nc.finalize()
    return nc


def _build_main(dp):
    """Phase B: streaming F-layer + batched segment softmax.
    feat is field-contiguous per supertile: every elementwise op runs on a
    flat [P, 2560] bf16 slice (DVE 2x mode); one product per supertile is
    placed on the otherwise-idle GpSimd engine."""
    nc = bacc.Bacc("TRN2", target_bir_lowering=False, debug=False,
                   num_devices=NCORES)
    FW2 = 2 * FW
    W2 = 2 * KK * D              # 2560: one field's width per supertile
    feat = nc.declare_dram_parameter("feat", [NST, P, FW2], bf16,
                                     isOutput=False)
    meta = nc.declare_dram_parameter("meta", [P, NT, 21], f32,
                                     isOutput=False)
    ta_ext = nc.declare_dram_parameter("ta", [P, NT * KK], f32, isOutput=True)

    S2 = 2

    with tile.TileContext(nc) as tc:
        with (
            tc.tile_pool(name="const", bufs=1) as cpool,
            tc.tile_pool(name="ld", bufs=3) as gpool,
            tc.tile_pool(name="mid", bufs=2) as mpool,
            tc.tile_pool(name="sm", bufs=2) as spool,
        ):
            logit_all = cpool.tile([P, NT, KK], f32)
            meta_sb = cpool.tile([P, NT, 21], f32)

            TT = nc.vector.tensor_tensor
            MU = mybir.AluOpType.mult
            AD = mybir.AluOpType.add

            K2 = S2 * KK             # 40 segment-slots per supertile

            for st in range(NST):
                # one full SBUF tile per field: fully-packed [P, 40, 64]
                # operands hit the fast DVE path (sliced views do not).
                # Loads ordered by first use in the op chain below.
                fts = {}
                for i in (1, 3, 2, 4, 5, 6, 0):
                    t = gpool.tile([P, W2], bf16, tag=f"fld{i}")
                    nc.sync.dma_start(out=t[:],
                                      in_=feat[st][:, i * W2:(i + 1) * W2])
                    fts[i] = t
                f1t = gpool.tile([P, S2 * D], bf16, tag="f1")
                nc.sync.dma_start(
                    out=f1t[:], in_=feat[st][:, 7 * W2:7 * W2 + S2 * D])
                if st == 0:
                    # meta is consumed only by the final softmax; keep it
                    # off the first supertile's critical load path
                    nc.sync.dma_start(out=meta_sb[:], in_=meta[:])

                def v3(t):
                    return t[:].rearrange("p (k d) -> p k d", d=D)

                f0, f3, f4, Av, Bv, Cv, Dv = (v3(fts[i]) for i in range(7))
                f1b = f1t[:].rearrange("p (s o d) -> p s o d", s=S2,
                                       d=D).to_broadcast([P, S2, KK, D])

                u1 = mpool.tile([P, W2], bf16, tag="u1")
                TT(out=v3(u1), in0=f3, in1=Av, op=MU)
                u2 = mpool.tile([P, W2], bf16, tag="u2")
                TT(out=v3(u2), in0=f4, in1=Bv, op=MU)
                u3 = mpool.tile([P, W2], bf16, tag="u3")
                TT(out=v3(u3), in0=f3, in1=Cv, op=MU)
                u4 = mpool.tile([P, W2], bf16, tag="u4")
                TT(out=v3(u4), in0=f4, in1=Dv, op=MU)
                TT(out=v3(u1), in0=v3(u1), in1=v3(u2), op=AD)
                TT(out=v3(u3), in0=v3(u3), in1=v3(u4), op=AD)
                TT(out=v3(u2), in0=v3(u1), in1=f0, op=MU)
                TT(out=u4[:].rearrange("p (s k d) -> p s k d", s=S2, d=D),
                   in0=u3[:].rearrange("p (s k d) -> p s k d", s=S2, d=D),
                   in1=f1b, op=MU)
                x = mpool.tile([P, W2], bf16, tag="x")
                TT(out=v3(x), in0=v3(u2), in1=v3(u4), op=AD)
                xr = mpool.tile([P, W2], bf16, tag="xr")
                nc.scalar.activation(out=xr[:], in_=x[:],
                                     func=mybir.ActivationFunctionType.Relu)

                xr4 = xr[:].rearrange("p (s k d) -> p s k d", s=S2, d=D)
                lsl = logit_all[:, 2 * st:2 * st + 2, :]
                if dp == D:
                    nc.vector.tensor_reduce(out=lsl, in_=xr4,
                                            axis=mybir.AxisListType.X, op=AD)
                elif dp == 0:
                    neg = spool.tile([P, S2, KK], f32, tag="neg")
                    nc.vector.tensor_reduce(out=neg[:], in_=xr4,
                                            axis=mybir.AxisListType.X, op=AD)
                    nc.vector.tensor_scalar_mul(lsl, neg[:], -1.0)
                else:
                    pos = spool.tile([P, S2, KK], f32, tag="pos")
                    nc.vector.tensor_reduce(out=pos[:], in_=xr4[:, :, :, 0:dp],
                                            axis=mybir.AxisListType.X, op=AD)
                    neg = spool.tile([P, S2, KK], f32, tag="neg")
                    nc.vector.tensor_reduce(out=neg[:], in_=xr4[:, :, :, dp:D],
                                            axis=mybir.AxisListType.X, op=AD)
                    nc.vector.scalar_tensor_tensor(
                        out=lsl, in0=pos[:], scalar=1.0, in1=neg[:],
                        op0=MU, op1=mybir.AluOpType.subtract)

            # batched segment softmax + attention weighting over all tiles
            m = spool.tile([P, NT], f32, tag="m")
            nc.vector.tensor_reduce(out=m[:], in_=logit_all[:],
                                    axis=mybir.AxisListType.X,
                                    op=mybir.AluOpType.max)
            mb = m[:].rearrange("p (t o) -> p t o", o=1)
            TT(out=logit_all[:], in0=logit_all[:],
               in1=mb.to_broadcast([P, NT, KK]),
               op=mybir.AluOpType.subtract)
            ex = cpool.tile([P, NT, KK], f32)
            nc.scalar.activation(out=ex[:], in_=logit_all[:],
                                 func=mybir.ActivationFunctionType.Exp)
            den = spool.tile([P, NT], f32, tag="den")
            nc.vector.tensor_reduce(out=den[:], in_=ex[:],
                                    axis=mybir.AxisListType.X, op=AD)
            rec = spool.tile([P, NT], f32, tag="rec")
            nc.vector.reciprocal(rec[:], den[:])
            sc = spool.tile([P, NT], f32, tag="sc")
            TT(out=sc[:], in0=rec[:],
               in1=meta_sb[:, :, 20:21].rearrange("p t o -> p (t o)"), op=MU)
            TT(out=ex[:], in0=ex[:], in1=meta_sb[:, :, 0:20], op=MU)
            scb = sc[:].rearrange("p (t o) -> p t o", o=1)
            TT(out=ex[:], in0=ex[:], in1=scb.to_broadcast([P, NT, KK]), op=MU)

            nc.sync.dma_start(
                out=ta_ext[:],
                in_=ex[:].rearrange("p a b -> p (a b)"))
    nc.finalize()
    return nc


_CACHE = {}


def _prep(inputs):
    """Host-side: permute the d axis by out_w sign, transpose/shard the
    projection inputs (pure data movement + integer index math)."""
    na = np.asarray(inputs["node_attention"], np.float32)
    se = np.asarray(inputs["scanned_edges"])
    ey = np.asarray(inputs["edges_y"], np.float32)
    huncon = np.asarray(inputs["hidden_uncon"], np.float32)[0]
    hcon = np.asarray(inputs["hidden_con"], np.float32)
    Wc = np.asarray(inputs["Wc"], np.float32)
    bc = np.asarray(inputs["bc"], np.float32)
    Wu = np.asarray(inputs["Wu"], np.float32)
    bu = np.asarray(inputs["bu"], np.float32)
    relt = np.asarray(inputs["rel_table"], np.float32)
    ws = np.asarray(inputs["ws"], np.float32)
    fb = np.asarray(inputs["fb"], np.float32)
    out_w = np.asarray(inputs["out_w"], np.float32)

    # d-permutation: positive out_w dims first
    perm = np.argsort(out_w <= 0, kind="stable")
    dp = int((out_w > 0).sum())
    Wcp = np.ascontiguousarray(Wc[:, perm]).astype(BF)
    Wup = np.ascontiguousarray(Wu[:, perm]).astype(BF)
    bp = np.empty((P, 2), np.float32)
    bp[0:D, 0] = bp[D:P, 0] = bc[perm]
    bp[0:D, 1] = bp[D:P, 1] = bu[perm]
    assert not np.any(fb != 0), "fb != 0 unsupported by this build"

    # fused per-rel tables ABCD[r] = [ws0+ws1*rel | ws2+ws3*rel |
    # ws4+ws5*rel | ws6+ws7*rel] * |out_w|  (parameter-table prep)
    wsp = ws[:, perm]
    absw = np.abs(out_w[perm])[None]
    rp = relt[:, perm]
    gtab = np.concatenate(
        [(wsp[2 * t] + wsp[2 * t + 1] * rp) * absw for t in range(4)],
        axis=1).astype(BF)                                       # [500, 256]

    eg, vi, vj, rel = (se[:, i].astype(np.int64) for i in range(4))
    e2vi, e2vj = se[:, 6].astype(np.int64), se[:, 7].astype(np.int64)

    hu_pad = np.zeros((HU_PAD, DLG), np.float32)
    hu_pad[:NN] = huncon
    in_maps_a = []
    for c in range(NCORES):
        hcT = np.ascontiguousarray(
            hcon[c * HC_SH:(c + 1) * HC_SH].T).astype(BF)
        huT = np.ascontiguousarray(
            hu_pad[c * HU_SH:(c + 1) * HU_SH].T).astype(BF)
        in_maps_a.append({"hconT": hcT, "huT": huT,
                          "wc_p": Wcp, "wu_p": Wup, "b_p": bp})
    return in_maps_a, dp, gtab, (na, eg, vi, vj, rel, e2vi, e2vj, ey)


def _pack_feats(hc_full, hu_full, gtab, host):
    """Host-side per-edge gather + packing into per-core feat/meta."""
    na, eg, vi, vj, rel, e2vi, e2vj, ey = host
    in_maps_b = []
    for c in range(NCORES):
        s = c * EPC
        fv = np.zeros((NST, P, 2 * FW), BF)

        def setf(off2, arr, w=KK * D):
            # sub-block s2 of field at off2 holds segment (2*st+s2)*128+p
            padded = np.zeros((SEG_PAD, w), BF)
            padded[:arr.shape[0]] = arr
            fv[:, :, off2:off2 + 2 * w] = padded.reshape(
                NST, 2, P, w).transpose(0, 2, 1, 3).reshape(NST, P, 2 * w)

        W1 = KK * D
        setf(0 * 2 * W1, hc_full[e2vi[s:s + EPC]].reshape(SEGS, W1))
        setf(1 * 2 * W1, hc_full[e2vj[s:s + EPC]].reshape(SEGS, W1))
        setf(2 * 2 * W1, hu_full[vj[s:s + EPC]].reshape(SEGS, W1))
        g_all = gtab[rel[s:s + EPC]]                 # [EPC, 256]
        for i in range(4):
            setf((3 + i) * 2 * W1,
                 np.ascontiguousarray(
                     g_all[:, i * D:(i + 1) * D]).reshape(SEGS, W1))
        setf(7 * 2 * W1, hu_full[vi[s:s + EPC][::KK]], w=D)

        mt = np.zeros((P, NT, 21), np.float32)
        eyp = np.zeros((SEG_PAD, KK), np.float32)
        eyp[:SEGS] = ey[s:s + EPC].reshape(SEGS, KK)
        mt[:, :, 0:20] = eyp.reshape(NT, P, KK).transpose(1, 0, 2)
        nav = np.zeros(SEG_PAD, np.float32)
        nav[:SEGS] = na[c // 2, vi[s:s + EPC][::KK]]
        mt[:, :, 20] = nav.reshape(NT, P).T
        in_maps_b.append({"feat": fv, "meta": mt})
    return in_maps_b


def kernel(**inputs):
    in_maps_a, dp, gtab, host = _prep(inputs)
    if "proj" not in _CACHE:
        _CACHE["proj"] = _build_proj()
    key = ("main", dp)
    if key not in _CACHE:
        _CACHE[key] = _build_main(dp)

    resA = run_bass_kernel_spmd(_CACHE["proj"], in_maps_a,
                                core_ids=list(range(NCORES)))
    hc_full = np.concatenate(
        [_unblock_hc(np.asarray(r["hc_sh"])) for r in resA.results], 0)
    hu_full = np.concatenate(
        [_unblock_hu(np.asarray(r["hu_sh"])) for r in resA.results], 0)

    in_maps_b = _pack_feats(hc_full, hu_full, gtab, host)
    resB = run_bass_kernel_spmd(_CACHE[key], in_maps_b,
                                core_ids=list(range(NCORES)))
    na, eg, vi, vj, rel, e2vi, e2vj, ey = host
    out = np.zeros((B, NN), np.float32)
    for c in range(NCORES):
        ta = np.asarray(resB.results[c]["ta"]).reshape(P, NT, KK)
        ta_edges = ta.transpose(1, 0, 2).reshape(-1)[:EPC]
        s = c * EPC
        np.add.at(out, (eg[s:s + EPC], vj[s:s + EPC]), ta_edges)
    return out


# revision 29
# speedup vs baseline: 1.1798x; 1.0212x over previous
"""Trainium2 8-core Bass kernel for nn_AttentionFlow (GNN message passing).

Strategy (per core c of 8):
  - Phase A (device): hc = tanh(hidden_con @ Wc + bc) and
    hu = tanh(hidden_uncon @ Wu + bu), row-sharded across the 8 cores.
    Host pre-transposes the inputs, so the device computes hc^T / hu^T
    directly: out[d', n] = sum_d W[d, d'] X^T[d, n] with the bias folded
    into the tanh activation (per-partition bias = d').  Two 512-column
    chunks share one [128, 512] PSUM tile (partition halves).
  - Host: gathers per-edge features from the phase-A tables
    (hc[e2vi], hc[e2vj], hu[vj], hu[vi_seg], ABCD[rel]) and packs them
    field-contiguously into a [10, 128, 2*FW] bf16 tensor per core (pure
    data movement / index math, no per-edge float compute).
  - Phase B (device): 8 streaming DMAs per 256-segment supertile (no
    indirect DMAs), one full SBUF tile per field so every DVE op runs on
    fully-packed [128, 40, 64] bf16 operands (fast path; sliced views of
    a larger tile run ~2x slower).  The F layer on DVE:
      x = f0*(f3*A + f4*B) + f1*(f3*C + f4*D)
    with ABCD[r] = [ws0+ws1*rel | ws2+ws3*rel | ws4+ws5*rel | ws6+ws7*rel]
    * |out_w| built host-side from the (tiny) parameter tables.
    logits = sum_d sign(w_d) relu(x_d): host permutes the d axis so
    positive-sign dims are contiguous -> two free-dim reduces.  Segment
    softmax (one segment of 20 edges per partition-row slot) is batched
    over all tiles at the end.  GpSimd is kept idle on purpose: its
    elementwise ops steal the shared DVE SBUF port and stall DVE.
  - Edges are sharded 50000/core, aligned to the 20-edge vi-segment
    structure, so the softmax is fully core-local.
  - Host: final (eg, vj) scatter-add of the per-edge trans_att partials.
"""

import sys

sys.path.insert(0, "/opt/trn_rl_repo")

import numpy as np
import ml_dtypes

from concourse import bass, bacc, mybir
import concourse.tile as tile
from concourse.bass_utils import run_bass_kernel_spmd

BF = ml_dtypes.bfloat16

NCORES = 8
B = 4
E = 400_000
EPC = E // NCORES            # 50000 edges per core
KK = 20                      # edges per vi segment
SEGS = EPC // KK             # 2500 segments per core
P = 128
NT = (SEGS + P - 1) // P     # 20 tiles of 128 segments
NST = NT // 2                # 10 supertiles of 256 segments
SEG_PAD = NT * P             # 2560
NN = 50_000
NREL = 500
D = 64
DLG = 256
NMEM = 131_072
HC_SH = NMEM // NCORES       # 16384 hidden_con rows per core
HU_SH = 7_168                # hidden_uncon rows per core (8*7168=57344)
HU_PAD = HU_SH * NCORES
FW = 9_024                   # feat cols: f0|f3|f4|A|B|C|D (7*1280) + f1 (64)

f32 = mybir.dt.float32
bf16 = mybir.dt.bfloat16


def _unblock_pairsT(buf, rows):
    """Device stores [128, 1024] bf16 tiles holding two 1024-row blocks
    (4 chunks of 512 cols of X^T; chunk = 4*B2 + 2*b + h, d on partition
    h*64+d).  Rebuild X^T [D, rows]."""
    n2 = rows // 2048
    b6 = buf.reshape(n2, 2, D, 2, 512)            # [B2, h, d, b, q]
    xT = b6.transpose(0, 3, 1, 2, 4).reshape(n2 * 4, D, 512)
    return xT.transpose(1, 0, 2).reshape(D, rows)


def _unblock_hc(buf):
    return _unblock_pairsT(buf, HC_SH).T


def _unblock_hu(buf):
    main = _unblock_pairsT(buf[:6144 * D], 6144)  # [64, 6144]
    tail = buf[6144 * D:].reshape(2, D, 512)      # [h, d, q]
    xT = np.concatenate([main, tail[0], tail[1]], axis=1)
    return xT.T                                    # [7168, 64]


def _build_proj():
    """Phase A: sharded hc/hu projections, transposed-output form."""
    nc = bacc.Bacc("TRN2", target_bir_lowering=False, debug=False,
                   num_devices=NCORES)
    hconT = nc.declare_dram_parameter("hconT", [D, HC_SH], bf16,
                                      isOutput=False)
    huT = nc.declare_dram_parameter("huT", [DLG, HU_SH], bf16, isOutput=False)
    wc_p = nc.declare_dram_parameter("wc_p", [D, D], bf16, isOutput=False)
    wu_p = nc.declare_dram_parameter("wu_p", [DLG, D], bf16, isOutput=False)
    b_p = nc.declare_dram_parameter("b_p", [P, 2], f32, isOutput=False)
    hc_out = nc.declare_dram_parameter("hc_sh", [HC_SH * D], bf16,
                                       isOutput=True)
    hu_out = nc.declare_dram_parameter("hu_sh", [HU_SH * D], bf16,
                                       isOutput=True)

    NCH = 4                       # input-load chunks for overlap
    CHC = HC_SH // NCH            # 4096
    CHU = HU_SH // NCH            # 1792

    with tile.TileContext(nc) as tc:
        with (
            tc.tile_pool(name="const", bufs=1) as cpool,
            tc.tile_pool(name="proj", bufs=3) as ppool,
            tc.tile_pool(name="psum", bufs=3, space="PSUM") as pspool,
        ):
            # weights/bias pre-cast to bf16 on the host: all loads go over
            # the two HWDGE rings (sync + scalar), no SWDGE involved
            wc_sb = cpool.tile([D, D], bf16)
            nc.scalar.dma_start(out=wc_sb[:], in_=wc_p[:])
            bb = cpool.tile([P, 2], f32)
            nc.scalar.dma_start(out=bb[:], in_=b_p[:])
            wu_sb = cpool.tile([DLG // 2, 2, D], bf16)
            nc.scalar.dma_start(out=wu_sb[:, 0, :], in_=wu_p[0:128, :])
            nc.scalar.dma_start(out=wu_sb[:, 1, :], in_=wu_p[128:256, :])

            # xt streams on the sync ring while xu streams on the scalar
            # ring; measured faster than sequencing both on one ring
            xt = cpool.tile([D, HC_SH], bf16)
            xu = cpool.tile([DLG // 2, 2, HU_SH], bf16)
            for ch in range(NCH):
                nc.sync.dma_start(out=xt[:, ch * CHC:(ch + 1) * CHC],
                                  in_=hconT[:, ch * CHC:(ch + 1) * CHC])
                nc.scalar.dma_start(out=xu[:, 0, ch * CHU:(ch + 1) * CHU],
                                    in_=huT[0:128, ch * CHU:(ch + 1) * CHU])
                nc.scalar.dma_start(out=xu[:, 1, ch * CHU:(ch + 1) * CHU],
                                    in_=huT[128:256, ch * CHU:(ch + 1) * CHU])

            def proj_block(blk, mms, bias, outbuf, ot2, half):
                ps = pspool.tile([P, 512], f32, space="PSUM", tag="ps")
                for h in range(2):
                    o = blk * 1024 + h * 512
                    for i, (lhs, rhs) in enumerate(mms):
                        nc.tensor.matmul(out=ps[h * D:(h + 1) * D, :],
                                         lhsT=lhs, rhs=rhs[:, o:o + 512],
                                         start=(i == 0),
                                         stop=(i == len(mms) - 1))
                nc.scalar.activation(out=ot2[:, half * 512:(half + 1) * 512],
                                     in_=ps[:],
                                     func=mybir.ActivationFunctionType.Tanh,
                                     bias=bias, scale=1.0)

            for blk2 in range(HC_SH // 2048):
                ot2 = ppool.tile([P, 1024], bf16, tag="ot2")
                for half in range(2):
                    proj_block(2 * blk2 + half, [(wc_sb[:], xt)],
                               bb[:, 0:1], hc_out, ot2, half)
                nc.sync.dma_start(
                    out=hc_out[blk2 * 2048 * D:(blk2 + 1) * 2048 * D],
                    in_=ot2[:])

            hu_mms = [(wu_sb[:, 0, :], xu[:, 0, :]), (wu_sb[:, 1, :], xu[:, 1, :])]
            for blk2 in range(HU_SH // 2048):
                ot2 = ppool.tile([P, 1024], bf16, tag="ot2")
                for half in range(2):
                    proj_block(2 * blk2 + half, hu_mms,
                               bb[:, 1:2], hu_out, ot2, half)
                nc.sync.dma_start(
                    out=hu_out[blk2 * 2048 * D:(blk2 + 1) * 2048 * D],
                    in_=ot2[:])
            # odd tail block of hu (7168 = 3*2048 + 1024)
            ot2 = ppool.tile([P, 1024], bf16, tag="ot2")
            proj_block(HU_SH // 1024 - 1, hu_mms, bb[:, 1:2], hu_out, ot2, 0)
            nc.sync.dma_start(
                out=hu_out[(HU_SH - 1024) * D:HU_SH * D],
                in_=ot2[:, 0:512])
    nc.finalize()
    return nc


def _build_main(dp):
    """Phase B: streaming F-layer + batched segment softmax.
    feat is field-contiguous per supertile: every elementwise op runs on a
    flat [P, 2560] bf16 slice (DVE 2x mode); one product per supertile is
    placed on the otherwise-idle GpSimd engine."""
    nc = bacc.Bacc("TRN2", target_bir_lowering=False, debug=False,
                   num_devices=NCORES)
    FW2 = 2 * FW
    W2 = 2 * KK * D              # 2560: one field's width per supertile
    feat = nc.declare_dram_parameter("feat", [NST, P, FW2], bf16,
                                     isOutput=False)
    meta = nc.declare_dram_parameter("meta", [P, NT, 21], f32,
                                     isOutput=False)
    ta_ext = nc.declare_dram_parameter("ta", [P, NT * KK], f32, isOutput=True)

    S2 = 2

    with tile.TileContext(nc) as tc:
        with (
            tc.tile_pool(name="const", bufs=1) as cpool,
            tc.tile_pool(name="ld", bufs=3) as gpool,
            tc.tile_pool(name="mid", bufs=2) as mpool,
            tc.tile_pool(name="sm", bufs=2) as spool,
        ):
            logit_all = cpool.tile([P, NT, KK], f32)
            meta_sb = cpool.tile([P, NT, 21], f32)
            ex_all = cpool.tile([P, NT, KK], f32)

            TT = nc.vector.tensor_tensor
            MU = mybir.AluOpType.mult
            AD = mybir.AluOpType.add

            def _softmax_range(t0, t1):
                # per-segment softmax + attention weighting for tiles t0:t1
                n = t1 - t0
                la = logit_all[:, t0:t1, :]
                exs = ex_all[:, t0:t1, :]
                m = spool.tile([P, n], f32, tag=f"m{t0}")
                nc.vector.tensor_reduce(out=m[:], in_=la,
                                        axis=mybir.AxisListType.X,
                                        op=mybir.AluOpType.max)
                mb = m[:].rearrange("p (t o) -> p t o", o=1)
                TT(out=la, in0=la, in1=mb.to_broadcast([P, n, KK]),
                   op=mybir.AluOpType.subtract)
                nc.scalar.activation(out=exs, in_=la,
                                     func=mybir.ActivationFunctionType.Exp)
                den = spool.tile([P, n], f32, tag=f"den{t0}")
                nc.vector.tensor_reduce(out=den[:], in_=exs,
                                        axis=mybir.AxisListType.X, op=AD)
                rec = spool.tile([P, n], f32, tag=f"rec{t0}")
                nc.vector.reciprocal(rec[:], den[:])
                sc = spool.tile([P, n], f32, tag=f"sc{t0}")
                TT(out=sc[:], in0=rec[:],
                   in1=meta_sb[:, t0:t1, 20:21].rearrange("p t o -> p (t o)"),
                   op=MU)
                TT(out=exs, in0=exs, in1=meta_sb[:, t0:t1, 0:20], op=MU)
                scb = sc[:].rearrange("p (t o) -> p t o", o=1)
                TT(out=exs, in0=exs, in1=scb.to_broadcast([P, n, KK]), op=MU)

            K2 = S2 * KK             # 40 segment-slots per supertile

            for st in range(NST):
                # one full SBUF tile per field: fully-packed [P, 40, 64]
                # operands hit the fast DVE path (sliced views do not).
                # Loads ordered by first use in the op chain below.
                fts = {}
                for i in (1, 3, 2, 4, 5, 6, 0):
                    t = gpool.tile([P, W2], bf16, tag=f"fld{i}")
                    nc.sync.dma_start(out=t[:],
                                      in_=feat[st][:, i * W2:(i + 1) * W2])
                    fts[i] = t
                f1t = gpool.tile([P, S2 * D], bf16, tag="f1")
                nc.sync.dma_start(
                    out=f1t[:], in_=feat[st][:, 7 * W2:7 * W2 + S2 * D])
                if st == 0:
                    # meta is consumed only by the final softmax; keep it
                    # off the first supertile's critical load path
                    nc.sync.dma_start(out=meta_sb[:], in_=meta[:])

                def v3(t):
                    return t[:].rearrange("p (k d) -> p k d", d=D)

                f0, f3, f4, Av, Bv, Cv, Dv = (v3(fts[i]) for i in range(7))
                f1b = f1t[:].rearrange("p (s o d) -> p s o d", s=S2,
                                       d=D).to_broadcast([P, S2, KK, D])

                u1 = mpool.tile([P, W2], bf16, tag="u1")
                TT(out=v3(u1), in0=f3, in1=Av, op=MU)
                u2 = mpool.tile([P, W2], bf16, tag="u2")
                TT(out=v3(u2), in0=f4, in1=Bv, op=MU)
                u3 = mpool.tile([P, W2], bf16, tag="u3")
                TT(out=v3(u3), in0=f3, in1=Cv, op=MU)
                u4 = mpool.tile([P, W2], bf16, tag="u4")
                TT(out=v3(u4), in0=f4, in1=Dv, op=MU)
                TT(out=v3(u1), in0=v3(u1), in1=v3(u2), op=AD)
                TT(out=v3(u3), in0=v3(u3), in1=v3(u4), op=AD)
                TT(out=v3(u2), in0=v3(u1), in1=f0, op=MU)
                TT(out=u4[:].rearrange("p (s k d) -> p s k d", s=S2, d=D),
                   in0=u3[:].rearrange("p (s k d) -> p s k d", s=S2, d=D),
                   in1=f1b, op=MU)
                x = mpool.tile([P, W2], bf16, tag="x")
                TT(out=v3(x), in0=v3(u2), in1=v3(u4), op=AD)
                xr = mpool.tile([P, W2], bf16, tag="xr")
                nc.scalar.activation(out=xr[:], in_=x[:],
                                     func=mybir.ActivationFunctionType.Relu)

                xr4 = xr[:].rearrange("p (s k d) -> p s k d", s=S2, d=D)
                lsl = logit_all[:, 2 * st:2 * st + 2, :]
                if dp == D:
                    nc.vector.tensor_reduce(out=lsl, in_=xr4,
                                            axis=mybir.AxisListType.X, op=AD)
                elif dp == 0:
                    neg = spool.tile([P, S2, KK], f32, tag="neg")
                    nc.vector.tensor_reduce(out=neg[:], in_=xr4,
                                            axis=mybir.AxisListType.X, op=AD)
                    nc.vector.tensor_scalar_mul(lsl, neg[:], -1.0)
                else:
                    pos = spool.tile([P, S2, KK], f32, tag="pos")
                    nc.vector.tensor_reduce(out=pos[:], in_=xr4[:, :, :, 0:dp],
                                            axis=mybir.AxisListType.X, op=AD)
                    neg = spool.tile([P, S2, KK], f32, tag="neg")
                    nc.vector.tensor_reduce(out=neg[:], in_=xr4[:, :, :, dp:D],
                                            axis=mybir.AxisListType.X, op=AD)
                    nc.vector.scalar_tensor_tensor(
                        out=lsl, in0=pos[:], scalar=1.0, in1=neg[:],
                        op0=MU, op1=mybir.AluOpType.subtract)

                if st == NST // 2 - 1:
                    # first-half softmax overlaps the remaining supertiles
                    _softmax_range(0, NT // 2)

            # second-half segment softmax (first half ran mid-kernel)
            _softmax_range(NT // 2, NT)

            nc.sync.dma_start(
                out=ta_ext[:],
                in_=ex_all[:].rearrange("p a b -> p (a b)"))
    nc.finalize()
    return nc


_CACHE = {}


def _prep(inputs):
    """Host-side: permute the d axis by out_w sign, transpose/shard the
    projection inputs (pure data movement + integer index math)."""
    na = np.asarray(inputs["node_attention"], np.float32)
    se = np.asarray(inputs["scanned_edges"])
    ey = np.asarray(inputs["edges_y"], np.float32)
    huncon = np.asarray(inputs["hidden_uncon"], np.float32)[0]
    hcon = np.asarray(inputs["hidden_con"], np.float32)
    Wc = np.asarray(inputs["Wc"], np.float32)
    bc = np.asarray(inputs["bc"], np.float32)
    Wu = np.asarray(inputs["Wu"], np.float32)
    bu = np.asarray(inputs["bu"], np.float32)
    relt = np.asarray(inputs["rel_table"], np.float32)
    ws = np.asarray(inputs["ws"], np.float32)
    fb = np.asarray(inputs["fb"], np.float32)
    out_w = np.asarray(inputs["out_w"], np.float32)

    # d-permutation: positive out_w dims first
    perm = np.argsort(out_w <= 0, kind="stable")
    dp = int((out_w > 0).sum())
    Wcp = np.ascontiguousarray(Wc[:, perm]).astype(BF)
    Wup = np.ascontiguousarray(Wu[:, perm]).astype(BF)
    bp = np.empty((P, 2), np.float32)
    bp[0:D, 0] = bp[D:P, 0] = bc[perm]
    bp[0:D, 1] = bp[D:P, 1] = bu[perm]
    assert not np.any(fb != 0), "fb != 0 unsupported by this build"

    # fused per-rel tables ABCD[r] = [ws0+ws1*rel | ws2+ws3*rel |
    # ws4+ws5*rel | ws6+ws7*rel] * |out_w|  (parameter-table prep)
    wsp = ws[:, perm]
    absw = np.abs(out_w[perm])[None]
    rp = relt[:, perm]
    gtab = np.concatenate(
        [(wsp[2 * t] + wsp[2 * t + 1] * rp) * absw for t in range(4)],
        axis=1).astype(BF)                                       # [500, 256]

    eg, vi, vj, rel = (se[:, i].astype(np.int64) for i in range(4))
    e2vi, e2vj = se[:, 6].astype(np.int64), se[:, 7].astype(np.int64)

    hu_pad = np.zeros((HU_PAD, DLG), np.float32)
    hu_pad[:NN] = huncon
    in_maps_a = []
    for c in range(NCORES):
        hcT = np.ascontiguousarray(
            hcon[c * HC_SH:(c + 1) * HC_SH].T).astype(BF)
        huT = np.ascontiguousarray(
            hu_pad[c * HU_SH:(c + 1) * HU_SH].T).astype(BF)
        in_maps_a.append({"hconT": hcT, "huT": huT,
                          "wc_p": Wcp, "wu_p": Wup, "b_p": bp})
    return in_maps_a, dp, gtab, (na, eg, vi, vj, rel, e2vi, e2vj, ey)


def _pack_feats(hc_full, hu_full, gtab, host):
    """Host-side per-edge gather + packing into per-core feat/meta."""
    na, eg, vi, vj, rel, e2vi, e2vj, ey = host
    in_maps_b = []
    for c in range(NCORES):
        s = c * EPC
        fv = np.zeros((NST, P, 2 * FW), BF)

        def setf(off2, arr, w=KK * D):
            # sub-block s2 of field at off2 holds segment (2*st+s2)*128+p
            padded = np.zeros((SEG_PAD, w), BF)
            padded[:arr.shape[0]] = arr
            fv[:, :, off2:off2 + 2 * w] = padded.reshape(
                NST, 2, P, w).transpose(0, 2, 1, 3).reshape(NST, P, 2 * w)

        W1 = KK * D
        setf(0 * 2 * W1, hc_full[e2vi[s:s + EPC]].reshape(SEGS, W1))
        setf(1 * 2 * W1, hc_full[e2vj[s:s + EPC]].reshape(SEGS, W1))
        setf(2 * 2 * W1, hu_full[vj[s:s + EPC]].reshape(SEGS, W1))
        g_all = gtab[rel[s:s + EPC]]                 # [EPC, 256]
        for i in range(4):
            setf((3 + i) * 2 * W1,
                 np.ascontiguousarray(
                     g_all[:, i * D:(i + 1) * D]).reshape(SEGS, W1))
        setf(7 * 2 * W1, hu_full[vi[s:s + EPC][::KK]], w=D)

        mt = np.zeros((P, NT, 21), np.float32)
        eyp = np.zeros((SEG_PAD, KK), np.float32)
        eyp[:SEGS] = ey[s:s + EPC].reshape(SEGS, KK)
        mt[:, :, 0:20] = eyp.reshape(NT, P, KK).transpose(1, 0, 2)
        nav = np.zeros(SEG_PAD, np.float32)
        nav[:SEGS] = na[c // 2, vi[s:s + EPC][::KK]]
        mt[:, :, 20] = nav.reshape(NT, P).T
        in_maps_b.append({"feat": fv, "meta": mt})
    return in_maps_b


def kernel(**inputs):
    in_maps_a, dp, gtab, host = _prep(inputs)
    if "proj" not in _CACHE:
        _CACHE["proj"] = _build_proj()
    key = ("main", dp)
    if key not in _CACHE:
        _CACHE[key] = _build_main(dp)

    resA = run_bass_kernel_spmd(_CACHE["proj"], in_maps_a,
                                core_ids=list(range(NCORES)))
    hc_full = np.concatenate(
        [_unblock_hc(np.asarray(r["hc_sh"])) for r in resA.results], 0)
    hu_full = np.concatenate(
        [_unblock_hu(np.asarray(r["hu_sh"])) for r in resA.results], 0)

    in_maps_b = _pack_feats(hc_full, hu_full, gtab, host)
    resB = run_bass_kernel_spmd(_CACHE[key], in_maps_b,
                                core_ids=list(range(NCORES)))
    na, eg, vi, vj, rel, e2vi, e2vj, ey = host
    out = np.zeros((B, NN), np.float32)
    for c in range(NCORES):
        ta = np.asarray(resB.results[c]["ta"]).reshape(P, NT, KK)
        ta_edges = ta.transpose(1, 0, 2).reshape(-1)[:EPC]
        s = c * EPC
        np.add.at(out, (eg[s:s + EPC], vj[s:s + EPC]), ta_edges)
    return out


# revision 30
# speedup vs baseline: 1.1915x; 1.0099x over previous
"""Trainium2 8-core Bass kernel for nn_AttentionFlow (GNN message passing).

Strategy (per core c of 8):
  - Phase A (device): hc = tanh(hidden_con @ Wc + bc) and
    hu = tanh(hidden_uncon @ Wu + bu), row-sharded across the 8 cores.
    Host pre-transposes the inputs, so the device computes hc^T / hu^T
    directly: out[d', n] = sum_d W[d, d'] X^T[d, n] with the bias folded
    into the tanh activation (per-partition bias = d').  Two 512-column
    chunks share one [128, 512] PSUM tile (partition halves).
  - Host: gathers per-edge features from the phase-A tables
    (hc[e2vi], hc[e2vj], hu[vj], hu[vi_seg], ABCD[rel]) and packs them
    field-contiguously into a [10, 128, 2*FW] bf16 tensor per core (pure
    data movement / index math, no per-edge float compute).
  - Phase B (device): 8 streaming DMAs per 256-segment supertile (no
    indirect DMAs), one full SBUF tile per field so every DVE op runs on
    fully-packed [128, 40, 64] bf16 operands (fast path; sliced views of
    a larger tile run ~2x slower).  The F layer on DVE:
      x = f0*(f3*A + f4*B) + f1*(f3*C + f4*D)
    with ABCD[r] = [ws0+ws1*rel | ws2+ws3*rel | ws4+ws5*rel | ws6+ws7*rel]
    * |out_w| built host-side from the (tiny) parameter tables.
    logits = sum_d sign(w_d) relu(x_d): host permutes the d axis so
    positive-sign dims are contiguous -> two free-dim reduces.  Segment
    softmax (one segment of 20 edges per partition-row slot) is batched
    over all tiles at the end.  GpSimd is kept idle on purpose: its
    elementwise ops steal the shared DVE SBUF port and stall DVE.
  - Edges are sharded 50000/core, aligned to the 20-edge vi-segment
    structure, so the softmax is fully core-local.
  - Host: final (eg, vj) scatter-add of the per-edge trans_att partials.
"""

import sys

sys.path.insert(0, "/opt/trn_rl_repo")

import numpy as np
import ml_dtypes

from concourse import bass, bacc, mybir
import concourse.tile as tile
from concourse.bass_utils import run_bass_kernel_spmd

BF = ml_dtypes.bfloat16

NCORES = 8
B = 4
E = 400_000
EPC = E // NCORES            # 50000 edges per core
KK = 20                      # edges per vi segment
SEGS = EPC // KK             # 2500 segments per core
P = 128
NT = (SEGS + P - 1) // P     # 20 tiles of 128 segments
NST = NT // 2                # 10 supertiles of 256 segments
SEG_PAD = NT * P             # 2560
NN = 50_000
NREL = 500
D = 64
DLG = 256
NMEM = 131_072
HC_SH = NMEM // NCORES       # 16384 hidden_con rows per core
HU_SH = 7_168                # hidden_uncon rows per core (8*7168=57344)
HU_PAD = HU_SH * NCORES
FW = 9_024                   # feat cols: f0|f3|f4|A|B|C|D (7*1280) + f1 (64)

f32 = mybir.dt.float32
bf16 = mybir.dt.bfloat16


def _unblock_pairsT(buf, rows):
    """Device stores [128, 1024] bf16 tiles holding two 1024-row blocks
    (4 chunks of 512 cols of X^T; chunk = 4*B2 + 2*b + h, d on partition
    h*64+d).  Rebuild X^T [D, rows]."""
    n2 = rows // 2048
    b6 = buf.reshape(n2, 2, D, 2, 512)            # [B2, h, d, b, q]
    xT = b6.transpose(0, 3, 1, 2, 4).reshape(n2 * 4, D, 512)
    return xT.transpose(1, 0, 2).reshape(D, rows)


def _unblock_hc(buf):
    return _unblock_pairsT(buf, HC_SH).T


def _unblock_hu(buf):
    main = _unblock_pairsT(buf[:6144 * D], 6144)  # [64, 6144]
    tail = buf[6144 * D:].reshape(2, D, 512)      # [h, d, q]
    xT = np.concatenate([main, tail[0], tail[1]], axis=1)
    return xT.T                                    # [7168, 64]


def _build_proj():
    """Phase A: sharded hc/hu projections, transposed-output form."""
    nc = bacc.Bacc("TRN2", target_bir_lowering=False, debug=False,
                   num_devices=NCORES)
    hconT = nc.declare_dram_parameter("hconT", [D, HC_SH], bf16,
                                      isOutput=False)
    huT = nc.declare_dram_parameter("huT", [DLG, HU_SH], bf16, isOutput=False)
    wc_p = nc.declare_dram_parameter("wc_p", [D, D], bf16, isOutput=False)
    wu_p = nc.declare_dram_parameter("wu_p", [DLG, D], bf16, isOutput=False)
    b_p = nc.declare_dram_parameter("b_p", [P, 2], f32, isOutput=False)
    hc_out = nc.declare_dram_parameter("hc_sh", [HC_SH * D], bf16,
                                       isOutput=True)
    hu_out = nc.declare_dram_parameter("hu_sh", [HU_SH * D], bf16,
                                       isOutput=True)

    NCH = 4                       # input-load chunks for overlap
    CHC = HC_SH // NCH            # 4096
    CHU = HU_SH // NCH            # 1792

    with tile.TileContext(nc) as tc:
        with (
            tc.tile_pool(name="const", bufs=1) as cpool,
            tc.tile_pool(name="proj", bufs=4) as ppool,
            tc.tile_pool(name="psum", bufs=4, space="PSUM") as pspool,
        ):
            # weights/bias pre-cast to bf16 on the host: all loads go over
            # the two HWDGE rings (sync + scalar), no SWDGE involved
            wc_sb = cpool.tile([D, D], bf16)
            nc.scalar.dma_start(out=wc_sb[:], in_=wc_p[:])
            bb = cpool.tile([P, 2], f32)
            nc.scalar.dma_start(out=bb[:], in_=b_p[:])
            wu_sb = cpool.tile([DLG // 2, 2, D], bf16)
            nc.scalar.dma_start(out=wu_sb[:, 0, :], in_=wu_p[0:128, :])
            nc.scalar.dma_start(out=wu_sb[:, 1, :], in_=wu_p[128:256, :])

            # xt streams on the sync ring while xu streams on the scalar
            # ring; measured faster than sequencing both on one ring
            xt = cpool.tile([D, HC_SH], bf16)
            xu = cpool.tile([DLG // 2, 2, HU_SH], bf16)
            nc.sync.dma_start(out=xt[:, 0:CHC // 2],
                              in_=hconT[:, 0:CHC // 2])
            for ch in range(NCH):
                lo = ch * CHC if ch else CHC // 2
                nc.sync.dma_start(out=xt[:, lo:(ch + 1) * CHC],
                                  in_=hconT[:, lo:(ch + 1) * CHC])
                nc.scalar.dma_start(out=xu[:, 0, ch * CHU:(ch + 1) * CHU],
                                    in_=huT[0:128, ch * CHU:(ch + 1) * CHU])
                nc.scalar.dma_start(out=xu[:, 1, ch * CHU:(ch + 1) * CHU],
                                    in_=huT[128:256, ch * CHU:(ch + 1) * CHU])

            def proj_block(blk, mms, bias, outbuf, ot2, half):
                ps = pspool.tile([P, 512], f32, space="PSUM", tag="ps")
                for h in range(2):
                    o = blk * 1024 + h * 512
                    for i, (lhs, rhs) in enumerate(mms):
                        nc.tensor.matmul(out=ps[h * D:(h + 1) * D, :],
                                         lhsT=lhs, rhs=rhs[:, o:o + 512],
                                         start=(i == 0),
                                         stop=(i == len(mms) - 1))
                nc.scalar.activation(out=ot2[:, half * 512:(half + 1) * 512],
                                     in_=ps[:],
                                     func=mybir.ActivationFunctionType.Tanh,
                                     bias=bias, scale=1.0)

            for blk2 in range(HC_SH // 2048):
                ot2 = ppool.tile([P, 1024], bf16, tag="ot2")
                for half in range(2):
                    proj_block(2 * blk2 + half, [(wc_sb[:], xt)],
                               bb[:, 0:1], hc_out, ot2, half)
                nc.sync.dma_start(
                    out=hc_out[blk2 * 2048 * D:(blk2 + 1) * 2048 * D],
                    in_=ot2[:])

            hu_mms = [(wu_sb[:, 0, :], xu[:, 0, :]), (wu_sb[:, 1, :], xu[:, 1, :])]
            for blk2 in range(HU_SH // 2048):
                ot2 = ppool.tile([P, 1024], bf16, tag="ot2")
                for half in range(2):
                    proj_block(2 * blk2 + half, hu_mms,
                               bb[:, 1:2], hu_out, ot2, half)
                nc.sync.dma_start(
                    out=hu_out[blk2 * 2048 * D:(blk2 + 1) * 2048 * D],
                    in_=ot2[:])
            # odd tail block of hu (7168 = 3*2048 + 1024)
            ot2 = ppool.tile([P, 1024], bf16, tag="ot2")
            proj_block(HU_SH // 1024 - 1, hu_mms, bb[:, 1:2], hu_out, ot2, 0)
            nc.sync.dma_start(
                out=hu_out[(HU_SH - 1024) * D:HU_SH * D],
                in_=ot2[:, 0:512])
    nc.finalize()
    return nc


def _build_main(dp):
    """Phase B: streaming F-layer + batched segment softmax.
    feat is field-contiguous per supertile: every elementwise op runs on a
    flat [P, 2560] bf16 slice (DVE 2x mode); one product per supertile is
    placed on the otherwise-idle GpSimd engine."""
    nc = bacc.Bacc("TRN2", target_bir_lowering=False, debug=False,
                   num_devices=NCORES)
    FW2 = 2 * FW
    W2 = 2 * KK * D              # 2560: one field's width per supertile
    feat = nc.declare_dram_parameter("feat", [NST, P, FW2], bf16,
                                     isOutput=False)
    meta = nc.declare_dram_parameter("meta", [P, NT, 21], f32,
                                     isOutput=False)
    ta_ext = nc.declare_dram_parameter("ta", [P, NT * KK], f32, isOutput=True)

    S2 = 2

    with tile.TileContext(nc) as tc:
        with (
            tc.tile_pool(name="const", bufs=1) as cpool,
            tc.tile_pool(name="ld", bufs=3) as gpool,
            tc.tile_pool(name="mid", bufs=2) as mpool,
            tc.tile_pool(name="sm", bufs=2) as spool,
        ):
            logit_all = cpool.tile([P, NT, KK], f32)
            meta_sb = cpool.tile([P, NT, 21], f32)
            ex_all = cpool.tile([P, NT, KK], f32)

            TT = nc.vector.tensor_tensor
            MU = mybir.AluOpType.mult
            AD = mybir.AluOpType.add

            def _softmax_range(t0, t1):
                # per-segment softmax + attention weighting for tiles t0:t1
                n = t1 - t0
                la = logit_all[:, t0:t1, :]
                exs = ex_all[:, t0:t1, :]
                m = spool.tile([P, n], f32, tag=f"m{t0}")
                nc.vector.tensor_reduce(out=m[:], in_=la,
                                        axis=mybir.AxisListType.X,
                                        op=mybir.AluOpType.max)
                mb = m[:].rearrange("p (t o) -> p t o", o=1)
                TT(out=la, in0=la, in1=mb.to_broadcast([P, n, KK]),
                   op=mybir.AluOpType.subtract)
                nc.scalar.activation(out=exs, in_=la,
                                     func=mybir.ActivationFunctionType.Exp)
                den = spool.tile([P, n], f32, tag=f"den{t0}")
                nc.vector.tensor_reduce(out=den[:], in_=exs,
                                        axis=mybir.AxisListType.X, op=AD)
                rec = spool.tile([P, n], f32, tag=f"rec{t0}")
                nc.vector.reciprocal(rec[:], den[:])
                sc = spool.tile([P, n], f32, tag=f"sc{t0}")
                TT(out=sc[:], in0=rec[:],
                   in1=meta_sb[:, t0:t1, 20:21].rearrange("p t o -> p (t o)"),
                   op=MU)
                TT(out=exs, in0=exs, in1=meta_sb[:, t0:t1, 0:20], op=MU)
                scb = sc[:].rearrange("p (t o) -> p t o", o=1)
                TT(out=exs, in0=exs, in1=scb.to_broadcast([P, n, KK]), op=MU)

            K2 = S2 * KK             # 40 segment-slots per supertile

            for st in range(NST):
                # one full SBUF tile per field: fully-packed [P, 40, 64]
                # operands hit the fast DVE path (sliced views do not).
                # Loads ordered by first use in the op chain below.
                fts = {}
                for i in (1, 3, 2, 4, 5, 6, 0):
                    t = gpool.tile([P, W2], bf16, tag=f"fld{i}")
                    nc.sync.dma_start(out=t[:],
                                      in_=feat[st][:, i * W2:(i + 1) * W2])
                    fts[i] = t
                f1t = gpool.tile([P, S2 * D], bf16, tag="f1")
                nc.sync.dma_start(
                    out=f1t[:], in_=feat[st][:, 7 * W2:7 * W2 + S2 * D])
                if st == 0:
                    # meta is consumed only by the final softmax; keep it
                    # off the first supertile's critical load path
                    nc.sync.dma_start(out=meta_sb[:], in_=meta[:])

                def v3(t):
                    return t[:].rearrange("p (k d) -> p k d", d=D)

                f0, f3, f4, Av, Bv, Cv, Dv = (v3(fts[i]) for i in range(7))
                f1b = f1t[:].rearrange("p (s o d) -> p s o d", s=S2,
                                       d=D).to_broadcast([P, S2, KK, D])

                u1 = mpool.tile([P, W2], bf16, tag="u1")
                TT(out=v3(u1), in0=f3, in1=Av, op=MU)
                u2 = mpool.tile([P, W2], bf16, tag="u2")
                TT(out=v3(u2), in0=f4, in1=Bv, op=MU)
                u3 = mpool.tile([P, W2], bf16, tag="u3")
                TT(out=v3(u3), in0=f3, in1=Cv, op=MU)
                u4 = mpool.tile([P, W2], bf16, tag="u4")
                TT(out=v3(u4), in0=f4, in1=Dv, op=MU)
                TT(out=v3(u1), in0=v3(u1), in1=v3(u2), op=AD)
                TT(out=v3(u3), in0=v3(u3), in1=v3(u4), op=AD)
                TT(out=v3(u2), in0=v3(u1), in1=f0, op=MU)
                TT(out=u4[:].rearrange("p (s k d) -> p s k d", s=S2, d=D),
                   in0=u3[:].rearrange("p (s k d) -> p s k d", s=S2, d=D),
                   in1=f1b, op=MU)
                x = mpool.tile([P, W2], bf16, tag="x")
                TT(out=v3(x), in0=v3(u2), in1=v3(u4), op=AD)
                xr = mpool.tile([P, W2], bf16, tag="xr")
                nc.scalar.activation(out=xr[:], in_=x[:],
                                     func=mybir.ActivationFunctionType.Relu)

                xr4 = xr[:].rearrange("p (s k d) -> p s k d", s=S2, d=D)
                lsl = logit_all[:, 2 * st:2 * st + 2, :]
                if dp == D:
                    nc.vector.tensor_reduce(out=lsl, in_=xr4,
                                            axis=mybir.AxisListType.X, op=AD)
                elif dp == 0:
                    neg = spool.tile([P, S2, KK], f32, tag="neg")
                    nc.vector.tensor_reduce(out=neg[:], in_=xr4,
                                            axis=mybir.AxisListType.X, op=AD)
                    nc.vector.tensor_scalar_mul(lsl, neg[:], -1.0)
                else:
                    pos = spool.tile([P, S2, KK], f32, tag="pos")
                    nc.vector.tensor_reduce(out=pos[:], in_=xr4[:, :, :, 0:dp],
                                            axis=mybir.AxisListType.X, op=AD)
                    neg = spool.tile([P, S2, KK], f32, tag="neg")
                    nc.vector.tensor_reduce(out=neg[:], in_=xr4[:, :, :, dp:D],
                                            axis=mybir.AxisListType.X, op=AD)
                    nc.vector.scalar_tensor_tensor(
                        out=lsl, in0=pos[:], scalar=1.0, in1=neg[:],
                        op0=MU, op1=mybir.AluOpType.subtract)

                if st == NST // 2 - 1:
                    # first-half softmax overlaps the remaining supertiles
                    _softmax_range(0, NT // 2)
                    nc.sync.dma_start(
                        out=ta_ext[:, 0:NT * KK // 2],
                        in_=ex_all[:, 0:NT // 2, :].rearrange(
                            "p a b -> p (a b)"))

            # second-half segment softmax (first half ran mid-kernel)
            _softmax_range(NT // 2, NT)

            nc.sync.dma_start(
                out=ta_ext[:, NT * KK // 2:],
                in_=ex_all[:, NT // 2:, :].rearrange("p a b -> p (a b)"))
    nc.finalize()
    return nc


_CACHE = {}


def _prep(inputs):
    """Host-side: permute the d axis by out_w sign, transpose/shard the
    projection inputs (pure data movement + integer index math)."""
    na = np.asarray(inputs["node_attention"], np.float32)
    se = np.asarray(inputs["scanned_edges"])
    ey = np.asarray(inputs["edges_y"], np.float32)
    huncon = np.asarray(inputs["hidden_uncon"], np.float32)[0]
    hcon = np.asarray(inputs["hidden_con"], np.float32)
    Wc = np.asarray(inputs["Wc"], np.float32)
    bc = np.asarray(inputs["bc"], np.float32)
    Wu = np.asarray(inputs["Wu"], np.float32)
    bu = np.asarray(inputs["bu"], np.float32)
    relt = np.asarray(inputs["rel_table"], np.float32)
    ws = np.asarray(inputs["ws"], np.float32)
    fb = np.asarray(inputs["fb"], np.float32)
    out_w = np.asarray(inputs["out_w"], np.float32)

    # d-permutation: positive out_w dims first
    perm = np.argsort(out_w <= 0, kind="stable")
    dp = int((out_w > 0).sum())
    Wcp = np.ascontiguousarray(Wc[:, perm]).astype(BF)
    Wup = np.ascontiguousarray(Wu[:, perm]).astype(BF)
    bp = np.empty((P, 2), np.float32)
    bp[0:D, 0] = bp[D:P, 0] = bc[perm]
    bp[0:D, 1] = bp[D:P, 1] = bu[perm]
    assert not np.any(fb != 0), "fb != 0 unsupported by this build"

    # fused per-rel tables ABCD[r] = [ws0+ws1*rel | ws2+ws3*rel |
    # ws4+ws5*rel | ws6+ws7*rel] * |out_w|  (parameter-table prep)
    wsp = ws[:, perm]
    absw = np.abs(out_w[perm])[None]
    rp = relt[:, perm]
    gtab = np.concatenate(
        [(wsp[2 * t] + wsp[2 * t + 1] * rp) * absw for t in range(4)],
        axis=1).astype(BF)                                       # [500, 256]

    eg, vi, vj, rel = (se[:, i].astype(np.int64) for i in range(4))
    e2vi, e2vj = se[:, 6].astype(np.int64), se[:, 7].astype(np.int64)

    hu_pad = np.zeros((HU_PAD, DLG), np.float32)
    hu_pad[:NN] = huncon
    in_maps_a = []
    for c in range(NCORES):
        hcT = np.ascontiguousarray(
            hcon[c * HC_SH:(c + 1) * HC_SH].T).astype(BF)
        huT = np.ascontiguousarray(
            hu_pad[c * HU_SH:(c + 1) * HU_SH].T).astype(BF)
        in_maps_a.append({"hconT": hcT, "huT": huT,
                          "wc_p": Wcp, "wu_p": Wup, "b_p": bp})
    return in_maps_a, dp, gtab, (na, eg, vi, vj, rel, e2vi, e2vj, ey)


def _pack_feats(hc_full, hu_full, gtab, host):
    """Host-side per-edge gather + packing into per-core feat/meta."""
    na, eg, vi, vj, rel, e2vi, e2vj, ey = host
    in_maps_b = []
    for c in range(NCORES):
        s = c * EPC
        fv = np.zeros((NST, P, 2 * FW), BF)

        def setf(off2, arr, w=KK * D):
            # sub-block s2 of field at off2 holds segment (2*st+s2)*128+p
            padded = np.zeros((SEG_PAD, w), BF)
            padded[:arr.shape[0]] = arr
            fv[:, :, off2:off2 + 2 * w] = padded.reshape(
                NST, 2, P, w).transpose(0, 2, 1, 3).reshape(NST, P, 2 * w)

        W1 = KK * D
        setf(0 * 2 * W1, hc_full[e2vi[s:s + EPC]].reshape(SEGS, W1))
        setf(1 * 2 * W1, hc_full[e2vj[s:s + EPC]].reshape(SEGS, W1))
        setf(2 * 2 * W1, hu_full[vj[s:s + EPC]].reshape(SEGS, W1))
        g_all = gtab[rel[s:s + EPC]]                 # [EPC, 256]
        for i in range(4):
            setf((3 + i) * 2 * W1,
                 np.ascontiguousarray(
                     g_all[:, i * D:(i + 1) * D]).reshape(SEGS, W1))
        setf(7 * 2 * W1, hu_full[vi[s:s + EPC][::KK]], w=D)

        mt = np.zeros((P, NT, 21), np.float32)
        eyp = np.zeros((SEG_PAD, KK), np.float32)
        eyp[:SEGS] = ey[s:s + EPC].reshape(SEGS, KK)
        mt[:, :, 0:20] = eyp.reshape(NT, P, KK).transpose(1, 0, 2)
        nav = np.zeros(SEG_PAD, np.float32)
        nav[:SEGS] = na[c // 2, vi[s:s + EPC][::KK]]
        mt[:, :, 20] = nav.reshape(NT, P).T
        in_maps_b.append({"feat": fv, "meta": mt})
    return in_maps_b


def kernel(**inputs):
    in_maps_a, dp, gtab, host = _prep(inputs)
    if "proj" not in _CACHE:
        _CACHE["proj"] = _build_proj()
    key = ("main", dp)
    if key not in _CACHE:
        _CACHE[key] = _build_main(dp)

    resA = run_bass_kernel_spmd(_CACHE["proj"], in_maps_a,
                                core_ids=list(range(NCORES)))
    hc_full = np.concatenate(
        [_unblock_hc(np.asarray(r["hc_sh"])) for r in resA.results], 0)
    hu_full = np.concatenate(
        [_unblock_hu(np.asarray(r["hu_sh"])) for r in resA.results], 0)

    in_maps_b = _pack_feats(hc_full, hu_full, gtab, host)
    resB = run_bass_kernel_spmd(_CACHE[key], in_maps_b,
                                core_ids=list(range(NCORES)))
    na, eg, vi, vj, rel, e2vi, e2vj, ey = host
    out = np.zeros((B, NN), np.float32)
    for c in range(NCORES):
        ta = np.asarray(resB.results[c]["ta"]).reshape(P, NT, KK)
        ta_edges = ta.transpose(1, 0, 2).reshape(-1)[:EPC]
        s = c * EPC
        np.add.at(out, (eg[s:s + EPC], vj[s:s + EPC]), ta_edges)
    return out
